# revision 22
# baseline (speedup 1.0000x reference)
"""Trainium2 Bass kernel for GQA attention (B=2, S=2048, D=2048, H=16, KVH=4).

Sharding: 8 cores = (batch b in {0,1}) x (kv-group g in {0..3}).
Each core: Q/K/V projections for its 4 q-heads + 1 kv head, RoPE, causal
softmax attention, and a partial output projection over its 512 Wo rows.
Host sums the 4 partials per batch.

On-device layout notes:
- x is passed per-core pre-transposed (xT [D, S]) so the contraction dim
  (D, then head_dim, then seq-k) is always the SBUF partition dim.
- Everything DMA'd / matmul'd is bf16 (fp32 PSUM accumulation); softmax
  normalization and RoPE temps stay fp32. rel-err ~4e-3 vs the fp32 ref.
- Wq/Wk columns are pre-permuted per head on host to deinterleave RoPE
  pairs (even dims -> rows 0:64, odd dims -> rows 64:128 of each head's
  Q^T/K^T block). The same permutation on Q and K preserves q.k dots.
- Scores are computed transposed (S^T [k, q]); the PV matmul consumes the
  exp'd scores directly (O^T = sum_k V[k,:]^T E^T[k,:]) so the probability
  matrix is never transposed. Softmax denominators: exp'd tiles are
  quad-merged on DVE and a [128,128] all-ones stationary matmul reduces
  each quad (4x fewer PE instructions than per-tile ones-matmuls); the
  quad matmuls are batched at the end of each (h, qc) so the PSUM bank is
  held only briefly. Normalized with approx reciprocal + multiply.
- Engine queues issue in program order, so emission order IS the per-
  engine schedule: scores run 3 k-tiles ahead of their PV matmuls (the
  exp on ACT paces the chain), next-chunk projections are woven between
  attention head-blocks, and the out-projection chunk follows as pure-PE
  filler. krot/qrot/vsb are parity-doubled so rep N+1's projections
  don't WAR-stall on rep N's attention reads.
"""

import math

import numpy as np

B = 2
S = 2048
D = 2048
N_HEADS = 16
N_KV_HEADS = 4
HD = 128  # head dim
G = 4  # kv groups (= heads per core group)
HPC = 4  # q heads per core
EQ = HPC * HD  # 512 q-proj cols per core
THETA = 10000.0
N_CORES = 8

SC = 4  # seq chunks of 512 in projections
QC = 4  # q chunks of 512 in attention
KT = 16  # k tiles of 128
DT = 16  # d tiles of 128
PRE = 4  # score matmuls emitted ahead of their PV matmuls

TINY_DMA = False  # probe-only: shrink input DMAs to measure compute-bound time


def _host_tables():
    """cos/sin tables [64, S] (transposed), deinterleave permutation [128]."""
    j = np.arange(HD // 2)
    inv_freq = 1.0 / THETA ** (2 * j / HD)  # [64]
    t = np.arange(S)
    ang = np.outer(inv_freq, t)  # [64, S]
    cosT = np.cos(ang).astype(np.float32)
    sinT = np.sin(ang).astype(np.float32)
    jj = np.arange(HD)
    perm = np.where(jj < 64, 2 * jj, 2 * (jj - 64) + 1)  # new row j <- old dim perm[j]
    return cosT, sinT, perm


def _host_masks():
    """Diagonal-subtile mask [128,128]: 1 where qq >= kk (causal-inclusive)."""
    kk = np.arange(128)[:, None]
    qq = np.arange(128)[None, :]
    return (qq >= kk).astype(np.float32)


def _emit_once(nc, tc, mybir, aps, shared, pools, parity=0, trim=True, split=True, first_rep=False, carry=None, last_rep=True):
    """One full forward pass, woven: B(0), then per sc: C(sc) x B(sc+1)
    slices, D(sc)."""
    f32 = mybir.dt.float32
    bf16 = mybir.dt.bfloat16
    AT = mybir.ActivationFunctionType
    xT, wq, wk, wv, wo, cosT, sinT, out = aps
    masks_sb, ones_sb, ident, cos_sb, sin_sb, qrotP, krotP, vsbP = shared[:8]
    qrot, krot, vsb = qrotP[parity], krotP[parity], vsbP[parity]
    inv_sqrt_hd = 1.0 / math.sqrt(HD)

    carry_out = {}

    def _ld(dst, src, eng=None):
        """DMA wrapper: in TINY_DMA probe mode, move only a 4-wide sliver.
        eng=None -> sync-engine HWDGE queue (565ns issue each); eng='pool'
        -> gpsimd SWDGE queue (cheap issue, keeps SP.SEQ clear)."""
        if TINY_DMA:
            dst = dst[..., 0:4]
            src = src[..., 0:4]
        (nc.gpsimd if eng == "pool" else nc.sync).dma_start(dst, src)

    (mpsum, wpool, xs_pool, tmp_pool, vt_pool, opool, et_pool, rd_pool,
     ostage_pool) = pools
    if True:
        # ---------------- input DMA preamble ----------------
        if carry is not None:
            wq_sb = carry["wq"]
            wk_sb = carry["wk"]
            wv_sb = carry["wv"]
        else:
            wq_sb = wpool.tile([128, DT, EQ], bf16, tag="wq", name="wq_sb")
            wk_sb = wpool.tile([128, DT, HD], bf16, tag="wk", name="wk_sb")
            wv_sb = wpool.tile([128, DT, HD], bf16, tag="wv", name="wv_sb")
        wo_sb = wpool.tile([128, HPC, D], bf16, tag="wo", name="wo_sb")
        # interleave the first seq-chunk's activation loads with the weight
        # slices so the d=0 matmuls un-gate early on a cold start; one DMA
        # instruction = one HW queue, so splitting also parallelizes.
        xchunk = {}
        wkr = wk.rearrange("(t p) e -> p t e", p=128)
        wvr = wv.rearrange("(t p) e -> p t e", p=128)
        wqr = wq.rearrange("(t p) e -> p t e", p=128)
        wor = wo.rearrange("(h p) e -> p h e", p=128)
        xTr = xT.rearrange("(t p) s -> p t s", p=128)
        if carry is not None:
            # this rep's wk/wv/wq/x0 loads were already emitted at the tail
            # of the previous rep (ahead of its D(3) stores on the SP queue)
            xchunk[0] = carry["xc0"]
            preloaded = True
        else:
            preloaded = False
            xc0 = xs_pool.tile([128, DT, 512], bf16, tag="xs", name="xs0")
            xchunk[0] = xc0
            _ld(wk_sb[:], wkr)
            # d=0 sliver first so the very first matmul un-gates in ~1us
            _ld(xc0[:, 0:1, :], xTr[:, 0:1, 0:512])
            _ld(xc0[:, 1:4, :], xTr[:, 1:4, 0:512])
        if first_rep:
            # consts ordered by first use: cos/sin for rope(kp0), mask for
            # the first attention block; tails after the hot loads
            masks_sb2, cos_sb2, sin_sb2, cosT2, sinT2, masks42 = shared[-1]
            nc.sync.dma_start(cos_sb2[:, 0:512], cosT2[:, 0:512])
            nc.sync.dma_start(sin_sb2[:, 0:512], sinT2[:, 0:512])
            nc.sync.dma_start(masks_sb2[:], masks42)
        if not preloaded:
            _ld(wv_sb[:], wvr)
            _ld(xc0[:, 4:10, :], xTr[:, 4:10, 0:512])
            _ld(wq_sb[:], wqr)
            _ld(xc0[:, 10:16, :], xTr[:, 10:16, 0:512])
        if first_rep:
            nc.sync.dma_start(cos_sb2[:, 512:S], cosT2[:, 512:S])
            nc.sync.dma_start(sin_sb2[:, 512:S], sinT2[:, 512:S])
        def prefetch_x(sc):
            xc = xs_pool.tile([128, DT, 512], bf16, tag="xs", name=f"xs{sc}")
            xchunk[sc] = xc
            _ld(xc[:], xTr[:, :, sc * 512 : (sc + 1) * 512])

        def _rope(src, dst, sc):
            c = cos_sb[:, sc * 512 : (sc + 1) * 512]
            s = sin_sb[:, sc * 512 : (sc + 1) * 512]
            t1 = tmp_pool.tile([64, 512], f32, tag="t1", name="t1")
            t2 = tmp_pool.tile([64, 512], f32, tag="t2", name="t2")
            t3 = tmp_pool.tile([64, 512], f32, tag="t3", name="t3")
            t4 = tmp_pool.tile([64, 512], f32, tag="t4", name="t4")
            nc.vector.tensor_mul(t1[:], src[0:64, :], c)
            nc.vector.tensor_mul(t2[:], src[64:128, :], s)
            nc.vector.tensor_mul(t3[:], src[0:64, :], s)
            nc.vector.tensor_mul(t4[:], src[64:128, :], c)
            nc.vector.tensor_sub(dst[0:64, :], t1[:], t2[:])
            nc.vector.tensor_add(dst[64:128, :], t3[:], t4[:])

        # -------- B slices: one projection target + its RoPE/evac --------
        def emit_kp(sc, first=False):
            kp = mpsum.tile([128, 512], f32, tag="kv", bufs=2, name="kp")
            for d in range(DT):
                if first and d == 0:
                    # split the very first matmul so it un-gates on a
                    # quarter-chunk of x instead of the full 256KB
                    nc.tensor.matmul(
                        kp[:, 0:256], wk_sb[:, 0, :], xchunk[sc][:, 0, 0:256],
                        start=True, stop=False,
                    )
                    nc.tensor.matmul(
                        kp[:, 256:512], wk_sb[:, 0, :], xchunk[sc][:, 0, 256:512],
                        start=False, stop=False,
                    )
                else:
                    nc.tensor.matmul(
                        kp[:], wk_sb[:, d, :], xchunk[sc][:, d, :],
                        start=d == 0, stop=d == DT - 1,
                    )
            _rope(kp, krot[sc], sc)

        def emit_vp(sc):
            vp = mpsum.tile([128, 512], f32, tag="kv", bufs=2, name="vp")
            for d in range(DT):
                nc.tensor.matmul(
                    vp[:], wv_sb[:, d, :], xchunk[sc][:, d, :],
                    start=d == 0, stop=d == DT - 1,
                )
            vT = vt_pool.tile([128, 512], bf16, tag="vT", name="vT")
            nc.scalar.copy(vT[:], vp[:])
            vtb = mpsum.tile([128, 512], bf16, tag="vtr", bufs=2, name="vtb")
            for t in range(4):
                nc.tensor.transpose(
                    vtb[:, t * 128 : (t + 1) * 128],
                    vT[:, t * 128 : (t + 1) * 128],
                    ident[:],
                )
                nc.scalar.copy(vsb[:, 4 * sc + t, :], vtb[:, t * 128 : (t + 1) * 128])

        def emit_qp(sc, h):
            qp = mpsum.tile([128, 512], f32, tag="qp", bufs=4, name=f"qp{h}")
            for d in range(DT):
                nc.tensor.matmul(
                    qp[:],
                    wq_sb[:, d, h * 128 : (h + 1) * 128],
                    xchunk[sc][:, d, :],
                    start=d == 0,
                    stop=d == DT - 1,
                )
            _rope(qp, qrot[(h, sc)], sc)

        def emit_B(sc):
            emit_kp(sc, first=sc == 0)
            emit_vp(sc)
            for h in range(HPC):
                emit_qp(sc, h)

        # -------- C: attention for one (h, qc) with score-ahead stagger --
        orot = {
            (h, c): opool.tile(
                [128, 512], bf16, tag=f"orot{h}_{c}", name=f"orot{h}_{c}"
            )
            for h in range(HPC)
            for c in range(QC)
        }

        def emit_attn(h, qc):
            nkt = 4 * (qc + 1)
            op = mpsum.tile([128, 512], f32, tag="vtr", bufs=2, name="op")
            accs = []
            pend = {}

            def emit_score(kt):
                # causal trim: diagonal k-tile (r>=0) only produces
                # q-columns >= 128*r of this chunk; skip the rest.
                r = kt - 4 * qc
                lo = 128 * r if (trim and r > 0) else 0
                w = slice(lo, 512)
                sp = mpsum.tile([128, 512], f32, tag="qp", bufs=4, name="sp")
                kc, ko = divmod(kt, 4)
                nc.tensor.matmul(
                    sp[:, w],
                    krot[kc][:, ko * 128 : (ko + 1) * 128],
                    qrot[(h, qc)][:, lo:512],
                    start=True,
                    stop=True,
                )
                if ko == 0:
                    # quad-first tile doubles as the quad accumulator
                    et = et_pool.tile([128, 512], bf16, tag="eacc", name="eacc")
                    accs.append(et)
                else:
                    et = et_pool.tile([128, 512], bf16, tag="et", name="et")
                nc.scalar.activation(et[:, w], sp[:, w], AT.Exp, scale=inv_sqrt_hd)
                if r >= 0:
                    # mask the [128,128] diagonal subtile only
                    nc.vector.tensor_mul(
                        et[:, lo : lo + 128], et[:, lo : lo + 128], masks_sb[:]
                    )
                pend[kt] = (et, w)

            for kt in range(min(PRE, nkt)):
                emit_score(kt)
            for kt in range(nkt):
                et, w = pend.pop(kt)
                nc.tensor.matmul(
                    op[:, w], vsb[:, kt, :], et[:, w],
                    start=kt == 0, stop=kt == nkt - 1,
                )
                if kt % 4 > 0:
                    acc = accs[kt // 4]
                    nc.vector.tensor_add(acc[:, w], acc[:, w], et[:, w])
                if kt + PRE < nkt:
                    emit_score(kt + PRE)
            # denominator: one ones-matmul per quad, batched so the PSUM
            # bank is held only briefly
            dp = mpsum.tile([128, 512], f32, tag="kv", bufs=2, name="dp")
            for i, acc in enumerate(accs):
                nc.tensor.matmul(
                    dp[:], ones_sb[:], acc[:], start=i == 0, stop=i == len(accs) - 1
                )
            rd = rd_pool.tile([128, 512], f32, tag="rd", name="rd")
            nc.vector.reciprocal_approx_fast(rd[:], dp[:])
            nc.vector.tensor_mul(orot[(h, qc)][:], op[:], rd[:])

        # -------- D: out-projection rows for one token chunk --------
        def emit_D(sc_):
            for st in range(4 * sc_, 4 * sc_ + 4):
                stsl = slice(st * 128, (st + 1) * 128)
                so = st % 4
                orow = ostage_pool.tile([128, D], bf16, tag="ost", name="ost")
                for mc in range(4):
                    msl = slice(mc * 512, (mc + 1) * 512)
                    pout = mpsum.tile([128, 512], f32, tag="qp", bufs=4, name="pout")
                    for h in range(HPC):
                        nc.tensor.matmul(
                            pout[:],
                            orot[(h, sc_)][:, so * 128 : (so + 1) * 128],
                            wo_sb[:, h, msl],
                            start=(h == 0),
                            stop=(h == HPC - 1),
                        )
                    nc.scalar.copy(orow[:, msl], pout[:])
                _ld(out[stsl, :], orow[:])

        # ---------------- woven schedule ----------------
        emit_B(0)
        for sc in range(SC):
            if sc < SC - 1:
                prefetch_x(sc + 1)
            if sc == 0:
                # wo isn't needed until D(0); queue it behind the x prefetch
                _ld(wo_sb[:], wor)
            for h in range(HPC):
                emit_attn(h, sc)
                if sc < SC - 1:
                    # weave one slice of the next chunk's projections into
                    # the ACT-paced attention stream
                    if h == 0:
                        emit_kp(sc + 1)
                    elif h == 1:
                        emit_vp(sc + 1)
                    elif h == 2:
                        emit_qp(sc + 1, 0)
                        emit_qp(sc + 1, 1)
                    else:
                        emit_qp(sc + 1, 2)
                        emit_qp(sc + 1, 3)
            if sc == SC - 1 and not last_rep:
                # next rep's input loads issue ahead of D(3)'s stores so the
                # waiting stores don't block them at the SP queue head
                nxc0 = xs_pool.tile([128, DT, 512], bf16, tag="xs", name="nxs0")
                nwk = wpool.tile([128, DT, HD], bf16, tag="wk", name="wk_sb")
                nwv = wpool.tile([128, DT, HD], bf16, tag="wv", name="wv_sb")
                nwq = wpool.tile([128, DT, EQ], bf16, tag="wq", name="wq_sb")
                _ld(nwk[:], wkr)
                _ld(nxc0[:, 0:1, :], xTr[:, 0:1, 0:512])
                _ld(nxc0[:, 1:4, :], xTr[:, 1:4, 0:512])
                _ld(nwv[:], wvr)
                _ld(nxc0[:, 4:10, :], xTr[:, 4:10, 0:512])
                _ld(nwq[:], wqr)
                _ld(nxc0[:, 10:16, :], xTr[:, 10:16, 0:512])
                carry_out.update(xc0=nxc0, wk=nwk, wv=nwv, wq=nwq)
            emit_D(sc)

    return carry_out


def _build_program(reps: int = 1, trim: bool = True, split: bool = True):
    import concourse.mybir as mybir
    import concourse.tile as tile
    from concourse import bacc
    from concourse.masks import make_identity

    f32 = mybir.dt.float32
    bf16 = mybir.dt.bfloat16

    nc = bacc.Bacc(
        "TRN2",
        target_bir_lowering=False,
        debug=False,
        enable_asserts=True,
        num_devices=N_CORES,
    )

    xT = nc.dram_tensor("xT", [D, S], bf16, kind="ExternalInput").ap()
    wq = nc.dram_tensor("wq", [D, EQ], bf16, kind="ExternalInput").ap()
    wk = nc.dram_tensor("wk", [D, HD], bf16, kind="ExternalInput").ap()
    wv = nc.dram_tensor("wv", [D, HD], bf16, kind="ExternalInput").ap()
    wo = nc.dram_tensor("wo", [EQ, D], bf16, kind="ExternalInput").ap()
    cosT = nc.dram_tensor("cosT", [64, S], f32, kind="ExternalInput").ap()
    sinT = nc.dram_tensor("sinT", [64, S], f32, kind="ExternalInput").ap()
    masks4 = nc.dram_tensor("masks4", [128, 128], bf16, kind="ExternalInput").ap()
    out = nc.dram_tensor("out", [S, D], bf16, kind="ExternalOutput").ap()
    aps = (xT, wq, wk, wv, wo, cosT, sinT, out)

    with tile.TileContext(nc) as tc:
        with (
            tc.tile_pool(name="persist", bufs=1) as persist,
            tc.tile_pool(name="consts", bufs=1) as consts,
            tc.tile_pool(name="mpsum", bufs=1, space="PSUM") as mpsum,
            tc.tile_pool(name="wpool", bufs=1) as wpool,
            tc.tile_pool(name="xs", bufs=2) as xs_pool,
            tc.tile_pool(name="ropetmp", bufs=1) as tmp_pool,
            tc.tile_pool(name="vtstage", bufs=2) as vt_pool,
            tc.tile_pool(name="opool", bufs=1) as opool,
            tc.tile_pool(name="et", bufs=6) as et_pool,
            tc.tile_pool(name="rd", bufs=2) as rd_pool,
            tc.tile_pool(name="ostage", bufs=2) as ostage_pool,
        ):
            pools = (
                mpsum, wpool, xs_pool, tmp_pool, vt_pool, opool, et_pool,
                rd_pool, ostage_pool,
            )
            qrotP, krotP, vsbP = [], [], []
            for p in range(2):
                qrotP.append(
                    {
                        (h, c): persist.tile(
                            [128, 512], bf16, tag=f"qrot{p}_{h}_{c}",
                            name=f"qrot{p}_{h}_{c}",
                        )
                        for h in range(HPC)
                        for c in range(SC)
                    }
                )
                krotP.append(
                    {
                        c: persist.tile(
                            [128, 512], bf16, tag=f"krot{p}_{c}", name=f"krot{p}_{c}"
                        )
                        for c in range(SC)
                    }
                )
                vsbP.append(
                    persist.tile([128, KT, HD], bf16, tag=f"vsb{p}", name=f"vsb{p}")
                )

            masks_sb = consts.tile([128, 128], bf16, tag="masks")
            ones_sb = consts.tile([128, 128], bf16, tag="ones")
            nc.gpsimd.memset(ones_sb[:], 1.0)
            ident = consts.tile([128, 128], bf16, tag="ident")
            make_identity(nc, ident[:])
            cos_sb = consts.tile([64, S], f32, tag="cos")
            sin_sb = consts.tile([64, S], f32, tag="sin")

            shared = (
                masks_sb, ones_sb, ident, cos_sb, sin_sb, qrotP, krotP, vsbP,
                (masks_sb, cos_sb, sin_sb, cosT, sinT, masks4),
            )
            carry = None
            for rep in range(reps):
                carry = _emit_once(
                    nc, tc, mybir, aps, shared, pools, parity=rep % 2,
                    trim=trim, split=split, first_rep=rep == 0, carry=carry,
                    last_rep=rep == reps - 1,
                )

    nc.compile()
    return nc


def _make_in_maps(x, Wq, Wk, Wv, Wo):
    import ml_dtypes

    bf16 = ml_dtypes.bfloat16
    cosT, sinT, perm = _host_tables()
    masks4 = _host_masks().astype(bf16)
    x = np.asarray(x, np.float32)
    Wq = np.asarray(Wq, np.float32)
    Wk = np.asarray(Wk, np.float32)
    Wv = np.asarray(Wv, np.float32)
    Wo = np.asarray(Wo, np.float32)

    # per-head column deinterleave for RoPE half-form
    qperm = np.concatenate([h * HD + perm for h in range(N_HEADS)])
    kperm = np.concatenate([h * HD + perm for h in range(N_KV_HEADS)])
    Wqp = Wq[:, qperm]
    Wkp = Wk[:, kperm]

    in_maps = []
    for core in range(N_CORES):
        b, g = divmod(core, G)
        in_maps.append(
            {
                "xT": np.ascontiguousarray(x[b].T).astype(bf16),
                "wq": np.ascontiguousarray(Wqp[:, g * EQ : (g + 1) * EQ]).astype(bf16),
                "wk": np.ascontiguousarray(Wkp[:, g * HD : (g + 1) * HD]).astype(bf16),
                "wv": np.ascontiguousarray(Wv[:, g * HD : (g + 1) * HD]).astype(bf16),
                "wo": np.ascontiguousarray(Wo[g * EQ : (g + 1) * EQ, :]).astype(bf16),
                "cosT": cosT,
                "sinT": sinT,
                "masks4": masks4,
            }
        )
    return in_maps


_CACHE = {}


def _get_program(reps: int = 1, trim: bool = True, split: bool = True):
    key = ("nc", reps, trim, split)
    if key not in _CACHE:
        _CACHE[key] = _build_program(reps, trim=trim, split=split)
    return _CACHE[key]


def kernel(x, mask, Wq, Wk, Wv, Wo):
    from concourse.bass_utils import run_bass_kernel_spmd

    nc = _get_program()
    in_maps = _make_in_maps(x, Wq, Wk, Wv, Wo)
    res = run_bass_kernel_spmd(nc, in_maps, core_ids=list(range(N_CORES)))
    parts = [res.results[c]["out"].astype(np.float32) for c in range(N_CORES)]
    out = np.stack(
        [
            parts[0] + parts[1] + parts[2] + parts[3],
            parts[4] + parts[5] + parts[6] + parts[7],
        ]
    ).astype(np.float32)
    return out


# revision 25
# speedup vs baseline: 1.0902x; 1.0902x over previous
"""Trainium2 Bass kernel for GQA attention (B=2, S=2048, D=2048, H=16, KVH=4).

Sharding: 8 cores = (batch b in {0,1}) x (kv-group g in {0..3}).
Each core: Q/K/V projections for its 4 q-heads + 1 kv head, RoPE, causal
softmax attention, and a partial output projection over its 512 Wo rows.
Host sums the 4 partials per batch.

On-device layout notes:
- x is passed per-core pre-transposed (xT [D, S]) so the contraction dim
  (D, then head_dim, then seq-k) is always the SBUF partition dim.
- Everything DMA'd / matmul'd is bf16 (fp32 PSUM accumulation); softmax
  normalization and RoPE temps stay fp32. rel-err ~4e-3 vs the fp32 ref.
- Wq/Wk columns are pre-permuted per head on host to deinterleave RoPE
  pairs (even dims -> rows 0:64, odd dims -> rows 64:128 of each head's
  Q^T/K^T block). The same permutation on Q and K preserves q.k dots.
- Scores are computed transposed (S^T [k, q]); the PV matmul consumes the
  exp'd scores directly (O^T = sum_k V[k,:]^T E^T[k,:]) so the probability
  matrix is never transposed. Softmax denominators: exp'd tiles are
  quad-merged on DVE and a [128,128] all-ones stationary matmul reduces
  each quad (4x fewer PE instructions than per-tile ones-matmuls); the
  quad matmuls are batched at the end of each (h, qc) so the PSUM bank is
  held only briefly. Normalized with approx reciprocal + multiply.
- Engine queues issue in program order, so emission order IS the per-
  engine schedule: scores run 3 k-tiles ahead of their PV matmuls (the
  exp on ACT paces the chain), next-chunk projections are woven between
  attention head-blocks, and the out-projection chunk follows as pure-PE
  filler. krot/qrot/vsb are parity-doubled so rep N+1's projections
  don't WAR-stall on rep N's attention reads.
"""

import math

import numpy as np

B = 2
S = 2048
D = 2048
N_HEADS = 16
N_KV_HEADS = 4
HD = 128  # head dim
G = 4  # kv groups (= heads per core group)
HPC = 4  # q heads per core
EQ = HPC * HD  # 512 q-proj cols per core
THETA = 10000.0
N_CORES = 8

SC = 4  # seq chunks of 512 in projections
QC = 4  # q chunks of 512 in attention
KT = 16  # k tiles of 128
DT = 16  # d tiles of 128
PRE = 3  # score matmuls emitted ahead of their PV matmuls

TINY_DMA = False  # probe-only: shrink input DMAs to measure compute-bound time


def _host_tables():
    """cos/sin tables [64, S] (transposed), deinterleave permutation [128]."""
    j = np.arange(HD // 2)
    inv_freq = 1.0 / THETA ** (2 * j / HD)  # [64]
    t = np.arange(S)
    ang = np.outer(inv_freq, t)  # [64, S]
    cosT = np.cos(ang).astype(np.float32)
    sinT = np.sin(ang).astype(np.float32)
    jj = np.arange(HD)
    perm = np.where(jj < 64, 2 * jj, 2 * (jj - 64) + 1)  # new row j <- old dim perm[j]
    return cosT, sinT, perm


def _host_masks():
    """Diagonal-subtile mask [128,128]: 1 where qq >= kk (causal-inclusive)."""
    kk = np.arange(128)[:, None]
    qq = np.arange(128)[None, :]
    return (qq >= kk).astype(np.float32)


def _emit_once(nc, tc, mybir, aps, shared, pools, parity=0, trim=True, split=True, first_rep=False, carry=None, last_rep=True):
    """One full forward pass, woven: B(0), then per sc: C(sc) x B(sc+1)
    slices, D(sc)."""
    f32 = mybir.dt.float32
    bf16 = mybir.dt.bfloat16
    AT = mybir.ActivationFunctionType
    xT, wq, wk, wv, wo, cosT, sinT, out = aps
    masks_sb, ones_sb, ident, cos_sb, sin_sb, qrotP, krotP, vsbP = shared[:8]
    qrot, krot, vsb = qrotP[parity], krotP[parity], vsbP[parity]
    inv_sqrt_hd = 1.0 / math.sqrt(HD)

    carry_out = {}

    def _ld(dst, src, eng=None):
        """DMA wrapper: in TINY_DMA probe mode, move only a 4-wide sliver.
        eng=None -> sync-engine HWDGE queue (565ns issue each); eng='pool'
        -> gpsimd SWDGE queue (cheap issue, keeps SP.SEQ clear)."""
        if TINY_DMA:
            dst = dst[..., 0:4]
            src = src[..., 0:4]
        (nc.gpsimd if eng == "pool" else nc.sync).dma_start(dst, src)

    (mpsum, wpool, xs_pool, tmp_pool, vt_pool, opool, et_pool, rd_pool,
     ostage_pool) = pools
    if True:
        # ---------------- input DMA preamble ----------------
        if carry is not None:
            wq_sb = carry["wq"]
            wk_sb = carry["wk"]
            wv_sb = carry["wv"]
        else:
            wq_sb = wpool.tile([128, DT, EQ], bf16, tag="wq", name="wq_sb")
            wk_sb = wpool.tile([128, DT, HD], bf16, tag="wk", name="wk_sb")
            wv_sb = wpool.tile([128, DT, HD], bf16, tag="wv", name="wv_sb")
        wo_sb = wpool.tile([128, HPC, D], bf16, tag="wo", name="wo_sb")
        # interleave the first seq-chunk's activation loads with the weight
        # slices so the d=0 matmuls un-gate early on a cold start; one DMA
        # instruction = one HW queue, so splitting also parallelizes.
        xchunk = {}
        wkr = wk.rearrange("(t p) e -> p t e", p=128)
        wvr = wv.rearrange("(t p) e -> p t e", p=128)
        wqr = wq.rearrange("(t p) e -> p t e", p=128)
        wor = wo.rearrange("(h p) e -> p h e", p=128)
        xTr = xT.rearrange("(t p) s -> p t s", p=128)
        if carry is not None:
            # this rep's wk/wv/wq/x0 loads were already emitted at the tail
            # of the previous rep (ahead of its D(3) stores on the SP queue)
            xchunk[0] = carry["xc0"]
            preloaded = True
        else:
            preloaded = False
            xc0 = xs_pool.tile([128, DT, 512], bf16, tag="xs", name="xs0")
            xchunk[0] = xc0
            _ld(wk_sb[:], wkr)
            # d=0 sliver first so the very first matmul un-gates in ~1us
            _ld(xc0[:, 0:1, :], xTr[:, 0:1, 0:512])
            _ld(xc0[:, 1:4, :], xTr[:, 1:4, 0:512])
        if first_rep:
            # consts ordered by first use: cos/sin for rope(kp0), mask for
            # the first attention block; tails after the hot loads
            masks_sb2, cos_sb2, sin_sb2, cosT2, sinT2, masks42 = shared[-1]
            nc.sync.dma_start(cos_sb2[:, 0:512], cosT2[:, 0:512])
            nc.sync.dma_start(sin_sb2[:, 0:512], sinT2[:, 0:512])
            nc.sync.dma_start(masks_sb2[:], masks42)
        if not preloaded:
            _ld(wv_sb[:], wvr)
            _ld(xc0[:, 4:10, :], xTr[:, 4:10, 0:512])
            _ld(wq_sb[:], wqr)
            _ld(xc0[:, 10:16, :], xTr[:, 10:16, 0:512])
        if first_rep:
            nc.sync.dma_start(cos_sb2[:, 512:S], cosT2[:, 512:S])
            nc.sync.dma_start(sin_sb2[:, 512:S], sinT2[:, 512:S])
        def prefetch_x(sc):
            xc = xs_pool.tile([128, DT, 512], bf16, tag="xs", name=f"xs{sc}")
            xchunk[sc] = xc
            _ld(xc[:], xTr[:, :, sc * 512 : (sc + 1) * 512])

        def _rope(src, dst, sc):
            c = cos_sb[:, sc * 512 : (sc + 1) * 512]
            s = sin_sb[:, sc * 512 : (sc + 1) * 512]
            t1 = tmp_pool.tile([64, 512], f32, tag="t1", name="t1")
            t2 = tmp_pool.tile([64, 512], f32, tag="t2", name="t2")
            t3 = tmp_pool.tile([64, 512], f32, tag="t3", name="t3")
            t4 = tmp_pool.tile([64, 512], f32, tag="t4", name="t4")
            nc.vector.tensor_mul(t1[:], src[0:64, :], c)
            nc.vector.tensor_mul(t2[:], src[64:128, :], s)
            nc.vector.tensor_mul(t3[:], src[0:64, :], s)
            nc.vector.tensor_mul(t4[:], src[64:128, :], c)
            nc.vector.tensor_sub(dst[0:64, :], t1[:], t2[:])
            nc.vector.tensor_add(dst[64:128, :], t3[:], t4[:])

        # -------- B slices: one projection target + its RoPE/evac --------
        def emit_kp(sc, first=False):
            kp = mpsum.tile([128, 512], f32, tag="kv", bufs=2, name="kp")
            for d in range(DT):
                if first and d == 0:
                    # split the very first matmul so it un-gates on a
                    # quarter-chunk of x instead of the full 256KB
                    nc.tensor.matmul(
                        kp[:, 0:256], wk_sb[:, 0, :], xchunk[sc][:, 0, 0:256],
                        start=True, stop=False,
                    )
                    nc.tensor.matmul(
                        kp[:, 256:512], wk_sb[:, 0, :], xchunk[sc][:, 0, 256:512],
                        start=False, stop=False,
                    )
                else:
                    nc.tensor.matmul(
                        kp[:], wk_sb[:, d, :], xchunk[sc][:, d, :],
                        start=d == 0, stop=d == DT - 1,
                    )
            _rope(kp, krot[sc], sc)

        def emit_vp(sc):
            vp = mpsum.tile([128, 512], f32, tag="kv", bufs=2, name="vp")
            for d in range(DT):
                nc.tensor.matmul(
                    vp[:], wv_sb[:, d, :], xchunk[sc][:, d, :],
                    start=d == 0, stop=d == DT - 1,
                )
            vT = vt_pool.tile([128, 512], bf16, tag="vT", name="vT")
            nc.scalar.copy(vT[:], vp[:])
            vtb = mpsum.tile([128, 512], bf16, tag="vtr", bufs=2, name="vtb")
            for t in range(4):
                nc.tensor.transpose(
                    vtb[:, t * 128 : (t + 1) * 128],
                    vT[:, t * 128 : (t + 1) * 128],
                    ident[:],
                )
                nc.scalar.copy(vsb[:, 4 * sc + t, :], vtb[:, t * 128 : (t + 1) * 128])

        def emit_qp(sc, h):
            qp = mpsum.tile([128, 512], f32, tag="qp", bufs=4, name=f"qp{h}")
            for d in range(DT):
                nc.tensor.matmul(
                    qp[:],
                    wq_sb[:, d, h * 128 : (h + 1) * 128],
                    xchunk[sc][:, d, :],
                    start=d == 0,
                    stop=d == DT - 1,
                )
            _rope(qp, qrot[(h, sc)], sc)

        def emit_B(sc):
            emit_kp(sc, first=sc == 0)
            emit_vp(sc)
            for h in range(HPC):
                emit_qp(sc, h)

        # -------- C: attention for one (h, qc) with score-ahead stagger --
        orot = {
            (h, c): opool.tile(
                [128, 512], bf16, tag=f"orot{h}_{c}", name=f"orot{h}_{c}"
            )
            for h in range(HPC)
            for c in range(QC)
        }

        def emit_attn(h, qc):
            nkt = 4 * (qc + 1)
            op = mpsum.tile([128, 512], f32, tag="vtr", bufs=2, name="op")
            accs = []
            pend = {}

            def emit_score(kt):
                # causal trim: diagonal k-tile (r>=0) only produces
                # q-columns >= 128*r of this chunk; skip the rest.
                r = kt - 4 * qc
                lo = 128 * r if (trim and r > 0) else 0
                w = slice(lo, 512)
                sp = mpsum.tile([128, 512], f32, tag="qp", bufs=4, name="sp")
                kc, ko = divmod(kt, 4)
                nc.tensor.matmul(
                    sp[:, w],
                    krot[kc][:, ko * 128 : (ko + 1) * 128],
                    qrot[(h, qc)][:, lo:512],
                    start=True,
                    stop=True,
                )
                if ko == 0:
                    # quad-first tile doubles as the quad accumulator
                    et = et_pool.tile([128, 512], bf16, tag="eacc", name="eacc")
                    accs.append(et)
                else:
                    et = et_pool.tile([128, 512], bf16, tag="et", name="et")
                nc.scalar.activation(et[:, w], sp[:, w], AT.Exp, scale=inv_sqrt_hd)
                if r >= 0:
                    # mask the [128,128] diagonal subtile only
                    nc.vector.tensor_mul(
                        et[:, lo : lo + 128], et[:, lo : lo + 128], masks_sb[:]
                    )
                pend[kt] = (et, w)

            for kt in range(min(PRE, nkt)):
                emit_score(kt)
            for kt in range(nkt):
                et, w = pend.pop(kt)
                nc.tensor.matmul(
                    op[:, w], vsb[:, kt, :], et[:, w],
                    start=kt == 0, stop=kt == nkt - 1,
                )
                if kt % 4 > 0:
                    acc = accs[kt // 4]
                    nc.vector.tensor_add(acc[:, w], acc[:, w], et[:, w])
                if kt + PRE < nkt:
                    emit_score(kt + PRE)
            # denominator: one ones-matmul per quad, batched so the PSUM
            # bank is held only briefly
            dp = mpsum.tile([128, 512], f32, tag="kv", bufs=2, name="dp")
            for i, acc in enumerate(accs):
                nc.tensor.matmul(
                    dp[:], ones_sb[:], acc[:], start=i == 0, stop=i == len(accs) - 1
                )
            rd = rd_pool.tile([128, 512], f32, tag="rd", name="rd")
            nc.vector.reciprocal_approx_fast(rd[:], dp[:])
            nc.vector.tensor_mul(orot[(h, qc)][:], op[:], rd[:])

        # -------- D: out-projection rows for one token chunk --------
        def emit_D(sc_, sts=None):
            for st in sts if sts is not None else range(4 * sc_, 4 * sc_ + 4):
                stsl = slice(st * 128, (st + 1) * 128)
                so = st % 4
                orow = ostage_pool.tile([128, D], bf16, tag="ost", name="ost")
                for mc in range(4):
                    msl = slice(mc * 512, (mc + 1) * 512)
                    pout = mpsum.tile([128, 512], f32, tag="kv", bufs=2, name="pout")
                    for h in range(HPC):
                        nc.tensor.matmul(
                            pout[:],
                            orot[(h, sc_)][:, so * 128 : (so + 1) * 128],
                            wo_sb[:, h, msl],
                            start=(h == 0),
                            stop=(h == HPC - 1),
                        )
                    nc.scalar.copy(orow[:, msl], pout[:])
                _ld(out[stsl, :], orow[:])

        # ---------------- woven schedule ----------------
        emit_B(0)
        for sc in range(SC):
            if sc < SC - 1:
                prefetch_x(sc + 1)
            if sc == 0:
                # wo isn't needed until D(0); queue it behind the x prefetch
                _ld(wo_sb[:], wor)
            for h in range(HPC):
                emit_attn(h, sc)
                if sc < SC - 1:
                    # weave one slice of the next chunk's projections into
                    # the ACT-paced attention stream
                    if h == 0:
                        emit_kp(sc + 1)
                    elif h == 1:
                        emit_vp(sc + 1)
                    elif h == 2:
                        emit_qp(sc + 1, 0)
                        emit_qp(sc + 1, 1)
                    else:
                        emit_qp(sc + 1, 2)
                        emit_qp(sc + 1, 3)
                if sc > 0:
                    # previous chunk's out-projection rows, one token-tile at
                    # a time: pure-PE filler for the ACT-paced attention
                    emit_D(sc - 1, sts=[4 * (sc - 1) + h])
            if sc == SC - 1 and not last_rep:
                # next rep's input loads issue ahead of D(3)'s stores so the
                # waiting stores don't block them at the SP queue head
                nxc0 = xs_pool.tile([128, DT, 512], bf16, tag="xs", name="nxs0")
                nwk = wpool.tile([128, DT, HD], bf16, tag="wk", name="wk_sb")
                nwv = wpool.tile([128, DT, HD], bf16, tag="wv", name="wv_sb")
                nwq = wpool.tile([128, DT, EQ], bf16, tag="wq", name="wq_sb")
                _ld(nwk[:], wkr)
                _ld(nxc0[:, 0:1, :], xTr[:, 0:1, 0:512])
                _ld(nxc0[:, 1:4, :], xTr[:, 1:4, 0:512])
                _ld(nwv[:], wvr)
                _ld(nxc0[:, 4:10, :], xTr[:, 4:10, 0:512])
                _ld(nwq[:], wqr)
                _ld(nxc0[:, 10:16, :], xTr[:, 10:16, 0:512])
                carry_out.update(xc0=nxc0, wk=nwk, wv=nwv, wq=nwq)
        emit_D(SC - 1)

    return carry_out


def _build_program(reps: int = 1, trim: bool = True, split: bool = True):
    import concourse.mybir as mybir
    import concourse.tile as tile
    from concourse import bacc
    from concourse.masks import make_identity

    f32 = mybir.dt.float32
    bf16 = mybir.dt.bfloat16

    nc = bacc.Bacc(
        "TRN2",
        target_bir_lowering=False,
        debug=False,
        enable_asserts=True,
        num_devices=N_CORES,
    )

    xT = nc.dram_tensor("xT", [D, S], bf16, kind="ExternalInput").ap()
    wq = nc.dram_tensor("wq", [D, EQ], bf16, kind="ExternalInput").ap()
    wk = nc.dram_tensor("wk", [D, HD], bf16, kind="ExternalInput").ap()
    wv = nc.dram_tensor("wv", [D, HD], bf16, kind="ExternalInput").ap()
    wo = nc.dram_tensor("wo", [EQ, D], bf16, kind="ExternalInput").ap()
    cosT = nc.dram_tensor("cosT", [64, S], f32, kind="ExternalInput").ap()
    sinT = nc.dram_tensor("sinT", [64, S], f32, kind="ExternalInput").ap()
    masks4 = nc.dram_tensor("masks4", [128, 128], bf16, kind="ExternalInput").ap()
    out = nc.dram_tensor("out", [S, D], bf16, kind="ExternalOutput").ap()
    aps = (xT, wq, wk, wv, wo, cosT, sinT, out)

    with tile.TileContext(nc) as tc:
        with (
            tc.tile_pool(name="persist", bufs=1) as persist,
            tc.tile_pool(name="consts", bufs=1) as consts,
            tc.tile_pool(name="mpsum", bufs=1, space="PSUM") as mpsum,
            tc.tile_pool(name="wpool", bufs=1) as wpool,
            tc.tile_pool(name="xs", bufs=2) as xs_pool,
            tc.tile_pool(name="ropetmp", bufs=1) as tmp_pool,
            tc.tile_pool(name="vtstage", bufs=2) as vt_pool,
            tc.tile_pool(name="opool", bufs=1) as opool,
            tc.tile_pool(name="et", bufs=8) as et_pool,
            tc.tile_pool(name="rd", bufs=2) as rd_pool,
            tc.tile_pool(name="ostage", bufs=2) as ostage_pool,
        ):
            pools = (
                mpsum, wpool, xs_pool, tmp_pool, vt_pool, opool, et_pool,
                rd_pool, ostage_pool,
            )
            qrotP, krotP, vsbP = [], [], []
            for p in range(2):
                qrotP.append(
                    {
                        (h, c): persist.tile(
                            [128, 512], bf16, tag=f"qrot{p}_{h}_{c}",
                            name=f"qrot{p}_{h}_{c}",
                        )
                        for h in range(HPC)
                        for c in range(SC)
                    }
                )
                krotP.append(
                    {
                        c: persist.tile(
                            [128, 512], bf16, tag=f"krot{p}_{c}", name=f"krot{p}_{c}"
                        )
                        for c in range(SC)
                    }
                )
                vsbP.append(
                    persist.tile([128, KT, HD], bf16, tag=f"vsb{p}", name=f"vsb{p}")
                )

            masks_sb = consts.tile([128, 128], bf16, tag="masks")
            ones_sb = consts.tile([128, 128], bf16, tag="ones")
            nc.gpsimd.memset(ones_sb[:], 1.0)
            ident = consts.tile([128, 128], bf16, tag="ident")
            make_identity(nc, ident[:])
            cos_sb = consts.tile([64, S], f32, tag="cos")
            sin_sb = consts.tile([64, S], f32, tag="sin")

            shared = (
                masks_sb, ones_sb, ident, cos_sb, sin_sb, qrotP, krotP, vsbP,
                (masks_sb, cos_sb, sin_sb, cosT, sinT, masks4),
            )
            carry = None
            for rep in range(reps):
                carry = _emit_once(
                    nc, tc, mybir, aps, shared, pools, parity=rep % 2,
                    trim=trim, split=split, first_rep=rep == 0, carry=carry,
                    last_rep=rep == reps - 1,
                )

    nc.compile()
    return nc


def _make_in_maps(x, Wq, Wk, Wv, Wo):
    import ml_dtypes

    bf16 = ml_dtypes.bfloat16
    cosT, sinT, perm = _host_tables()
    masks4 = _host_masks().astype(bf16)
    x = np.asarray(x, np.float32)
    Wq = np.asarray(Wq, np.float32)
    Wk = np.asarray(Wk, np.float32)
    Wv = np.asarray(Wv, np.float32)
    Wo = np.asarray(Wo, np.float32)

    # per-head column deinterleave for RoPE half-form
    qperm = np.concatenate([h * HD + perm for h in range(N_HEADS)])
    kperm = np.concatenate([h * HD + perm for h in range(N_KV_HEADS)])
    Wqp = Wq[:, qperm]
    Wkp = Wk[:, kperm]

    in_maps = []
    for core in range(N_CORES):
        b, g = divmod(core, G)
        in_maps.append(
            {
                "xT": np.ascontiguousarray(x[b].T).astype(bf16),
                "wq": np.ascontiguousarray(Wqp[:, g * EQ : (g + 1) * EQ]).astype(bf16),
                "wk": np.ascontiguousarray(Wkp[:, g * HD : (g + 1) * HD]).astype(bf16),
                "wv": np.ascontiguousarray(Wv[:, g * HD : (g + 1) * HD]).astype(bf16),
                "wo": np.ascontiguousarray(Wo[g * EQ : (g + 1) * EQ, :]).astype(bf16),
                "cosT": cosT,
                "sinT": sinT,
                "masks4": masks4,
            }
        )
    return in_maps


_CACHE = {}


def _get_program(reps: int = 1, trim: bool = True, split: bool = True):
    key = ("nc", reps, trim, split)
    if key not in _CACHE:
        _CACHE[key] = _build_program(reps, trim=trim, split=split)
    return _CACHE[key]


def kernel(x, mask, Wq, Wk, Wv, Wo):
    from concourse.bass_utils import run_bass_kernel_spmd

    nc = _get_program()
    in_maps = _make_in_maps(x, Wq, Wk, Wv, Wo)
    res = run_bass_kernel_spmd(nc, in_maps, core_ids=list(range(N_CORES)))
    parts = [res.results[c]["out"].astype(np.float32) for c in range(N_CORES)]
    out = np.stack(
        [
            parts[0] + parts[1] + parts[2] + parts[3],
            parts[4] + parts[5] + parts[6] + parts[7],
        ]
    ).astype(np.float32)
    return out
